# revision 10
# baseline (speedup 1.0000x reference)
"""Trainium2 Bass kernel for nn_Cont_Loss_21930103014244.

Computes: loss = sum over (b, c, j_even, h, w) of
    (out[b,c,2j,h,w] - target[b,c,2j+1,h,w])^2 / (32*128*128 * 8)

Strategy (data-parallel over batch, B=8 -> one batch element per core):
  - Only half of each input participates (even-j slices of `out`, odd-j
    slices of `target`). The host stages exactly that half per core,
    compacted to [2, 128, 16384] (row r = g*128 + p <-> (c, j_idx) =
    divmod(r, 8)) and cast to float16. The f32->fp16 quantization
    perturbs this loss by ~3e-7 relative (measured; gate is 2e-2) and
    halves HBM traffic, which is the binding constraint: all 8 cores
    streaming together saturate ~2.7 TB/s of chip HBM bandwidth.
  - The kernel streams o/t chunks [128, w] HBM->SBUF, computes d = o - t
    on VectorE (fp16, in place), then Square+accumulate(f32) on ScalarE
    (activation(Square, accum_out=...)) giving per-partition partials.
  - The last chunks ramp down in width (2048 -> 256 cols) so the serial
    tail after the final DMA (sub -> square) is short.
  - Per-core output: [128, nchunks] f32 partial sums; host reduces and
    scales in f64.
"""

import numpy as np

_CACHE = {}

B, C, W, H, Wd = 8, 32, 16, 128, 128
_COLS = H * Wd           # 16384 elements per row
_F = 4096                # main chunk width
_BUFS = 12               # buffers per io tile tag
_RAMP = (2048, 1024, 512, 256, 256)  # tail chunk widths (sum = _F)
_ACCW = 2048             # max cols per CCE accum-DMA descriptor (cayman limit)
_IN_DT = "float8e4"      # staged dtype: float16 | bfloat16 | float8e4 | float32
_CCE = True              # form d = o + (-t) via SWDGE accum-DMA (CCE add);
                         # requires _IN_DT staging of o and NEGATED t.
                         # Falls back to DVE subtract when False.
_SCALE = 1.0 / (C * H * Wd * (W // 2))


def _np_dt(name):
    if name == "float16":
        return np.float16
    if name == "float32":
        return np.float32
    import ml_dtypes

    if name == "bfloat16":
        return np.dtype(ml_dtypes.bfloat16)
    if name == "float8e4":
        return np.dtype(ml_dtypes.float8_e4m3)
    if name == "float8e3":
        return np.dtype(ml_dtypes.float8_e3m4)
    raise ValueError(name)


def _bir_dt(mybir, name):
    return getattr(mybir.dt, name)


def _plan(F=_F, ramp=_RAMP):
    """Per row-group list of (col_start, width). The final chunks of the
    last row-group ramp down so the post-last-DMA serial tail (subtract ->
    square -> output) is short."""
    assert ramp == () or sum(ramp) == F
    plans = []
    for g in range(2):
        cols = []
        if g == 1 and ramp:
            main = _COLS - F
            cols += [(c, F) for c in range(0, main, F)]
            c = main
            for w in ramp:
                cols.append((c, w))
                c += w
        else:
            cols = [(c, F) for c in range(0, _COLS, F)]
        plans.append(cols)
    return plans


def _nacc(plans):
    return sum(len(p) for p in plans)


def _emit_body(nc, io_pool, acc, o, t, plans, F, t_dma, compute, r,
               in_dt=_IN_DT, d_pool=None, cce=_CCE):
    """One full pass over the staged halves.

    cce=True (fp8 path): per chunk, one tile receives o via HWDGE, then the
    staged NEGATED t via a SWDGE accum-DMA (CCE add) so the tile holds
    d = o - t with no vector-engine work; ACT squares it in place with a
    f32 accum column per chunk.

    cce=False: DMA o and t into separate tiles, d = o - t on DVE (in place,
    or into a d_pool fp16 tile for 1-byte dtypes), then ACT Square+accum."""
    import concourse.mybir as mybir

    dt_in = _bir_dt(mybir, in_dt)
    f16 = mybir.dt.float16
    ai = 0
    for g in range(2):
        for k, (c0, w) in enumerate(plans[g]):
            if cce:
                d_t = io_pool.tile(
                    [128, w], dt_in, tag="io", name=f"io{r}_{g}_{k}",
                    padded_shape=[128, F],
                )
                nc.sync.dma_start(d_t[:], o[g, :, c0 : c0 + w])
                # CCE accumulate caps at 2048 elements per descriptor
                # (cayman); slice the accum-DMA accordingly.
                for a0 in range(0, w, _ACCW):
                    aw = min(_ACCW, w - a0)
                    nc.gpsimd.dma_start(
                        d_t[:, a0 : a0 + aw],
                        t[g, :, c0 + a0 : c0 + a0 + aw],
                        accum_op=mybir.AluOpType.add,
                    )
                if compute:
                    nc.scalar.activation(
                        d_t[:],
                        d_t[:],
                        mybir.ActivationFunctionType.Square,
                        accum_out=acc[:, ai : ai + 1],
                    )
                ai += 1
                continue
            o_t = io_pool.tile(
                [128, w], dt_in, tag="o", name=f"ot{r}_{g}_{k}",
                padded_shape=[128, F],
            )
            t_t = io_pool.tile(
                [128, w], dt_in, tag="t", name=f"tt{r}_{g}_{k}",
                padded_shape=[128, F],
            )
            nc.sync.dma_start(o_t[:], o[g, :, c0 : c0 + w])
            t_dma.dma_start(t_t[:], t[g, :, c0 : c0 + w])
            if compute:
                if d_pool is not None:
                    d_t = d_pool.tile(
                        [128, w], f16, tag="d", name=f"dt{r}_{g}_{k}",
                        padded_shape=[128, F],
                    )
                else:
                    d_t = t_t
                nc.vector.tensor_sub(d_t[:], o_t[:], t_t[:])
                nc.scalar.activation(
                    d_t[:],
                    d_t[:],
                    mybir.ActivationFunctionType.Square,
                    accum_out=acc[:, ai : ai + 1],
                )
            ai += 1


def _build_module(
    reps=1,
    F=_F,
    bufs=_BUFS,
    ramp=_RAMP,
    split_rings=False,
    compute=True,
    in_dt=_IN_DT,
    cce=_CCE,
):
    import concourse.bacc as bacc
    import concourse.mybir as mybir
    from concourse import tile

    f32 = mybir.dt.float32
    dt_in = _bir_dt(mybir, in_dt)
    plans = _plan(F, ramp)
    nacc = _nacc(plans)
    one_byte = mybir.dt.size(dt_in) == 1
    nc = bacc.Bacc("TRN2", target_bir_lowering=False, debug=False, num_devices=B)

    o = nc.dram_tensor("o", [2, 128, _COLS], dt_in, kind="ExternalInput").ap()
    t = nc.dram_tensor("t", [2, 128, _COLS], dt_in, kind="ExternalInput").ap()
    partials = nc.dram_tensor(
        "partials", [128, nacc], f32, kind="ExternalOutput"
    ).ap()

    with tile.TileContext(nc) as tc:
        with (
            tc.tile_pool(name="io", bufs=bufs) as io_pool,
            tc.tile_pool(name="d", bufs=(bufs if one_byte else 1)) as d_pool,
            tc.tile_pool(name="misc", bufs=1) as misc,
        ):
            acc = misc.tile([128, nacc], f32, name="acc")
            if not compute:
                # acc never written by compute; zero it so output is defined
                nc.vector.memset(acc[:], 0.0)
            t_dma = nc.scalar if split_rings else nc.sync
            for r in range(reps):
                _emit_body(
                    nc, io_pool, acc, o, t, plans, F, t_dma, compute, r,
                    in_dt=in_dt, cce=cce,
                    d_pool=(d_pool if one_byte and not cce else None),
                )
            nc.sync.dma_start(partials[:], acc[:])

    nc.compile()
    return nc


def _build_loop_module(
    R,
    F=_F,
    bufs=_BUFS,
    ramp=_RAMP,
    split_rings=False,
    compute=True,
    in_dt=_IN_DT,
    cce=_CCE,
):
    """Same pipeline wrapped in a hardware For_i loop, for wall-clock timing:
    R iterations inside one NEFF make device time >> host dispatch noise.
    The back-edge barrier (~2us) makes this a slight over-estimate per iter."""
    import concourse.bacc as bacc
    import concourse.mybir as mybir
    from concourse import tile

    f32 = mybir.dt.float32
    dt_in = _bir_dt(mybir, in_dt)
    plans = _plan(F, ramp)
    nacc = _nacc(plans)
    one_byte = mybir.dt.size(dt_in) == 1
    nc = bacc.Bacc("TRN2", target_bir_lowering=False, debug=False, num_devices=B)

    o = nc.dram_tensor("o", [2, 128, _COLS], dt_in, kind="ExternalInput").ap()
    t = nc.dram_tensor("t", [2, 128, _COLS], dt_in, kind="ExternalInput").ap()
    partials = nc.dram_tensor(
        "partials", [128, nacc], f32, kind="ExternalOutput"
    ).ap()

    with tile.TileContext(nc) as tc:
        with (
            tc.tile_pool(name="io", bufs=bufs) as io_pool,
            tc.tile_pool(name="d", bufs=(bufs if one_byte else 1)) as d_pool,
            tc.tile_pool(name="misc", bufs=1) as misc,
        ):
            acc = misc.tile([128, nacc], f32, name="acc")
            if not compute:
                nc.vector.memset(acc[:], 0.0)
            t_dma = nc.scalar if split_rings else nc.sync

            with tc.For_i(0, R, 1):
                _emit_body(
                    nc, io_pool, acc, o, t, plans, F, t_dma, compute, 0,
                    in_dt=in_dt, cce=cce,
                    d_pool=(d_pool if one_byte and not cce else None),
                )
            nc.sync.dma_start(partials[:], acc[:])

    nc.compile()
    return nc


class _Executor:
    """Persistent PJRT executor over the 8 axon-tunneled NeuronCores.

    Mirrors concourse.bass2jax.run_bass_via_pjrt's multi-core path but keeps
    the jitted callable and on-device inputs alive so repeated executions
    don't re-stage inputs over the tunnel (and so timing loops measure only
    dispatch + device execution).
    """

    def __init__(self, nc, n_cores):
        import concourse.mybir as mybir
        import jax
        from jax.sharding import Mesh, NamedSharding, PartitionSpec
        from concourse.bass2jax import (
            _bass_exec_p,
            install_neuronx_cc_hook,
            partition_id_tensor,
        )

        try:
            from jax.experimental.shard_map import shard_map
        except ImportError:
            from jax import shard_map

        install_neuronx_cc_hook()
        assert nc.dbg_addr is None
        partition_name = (
            nc.partition_id_tensor.name if nc.partition_id_tensor else None
        )

        in_names, out_names, out_avals, zero_outs = [], [], [], []
        for alloc in nc.m.functions[0].allocations:
            if not isinstance(alloc, mybir.MemoryLocationSet):
                continue
            name = alloc.memorylocations[0].name
            if alloc.kind == "ExternalInput":
                if name != partition_name:
                    in_names.append(name)
            elif alloc.kind == "ExternalOutput":
                shape = tuple(alloc.tensor_shape)
                dtype = mybir.dt.np(alloc.dtype)
                out_names.append(name)
                out_avals.append(jax.core.ShapedArray(shape, dtype))
                zero_outs.append(np.zeros(shape, dtype))

        self.jax = jax
        self.in_names = list(in_names)
        self.out_names = out_names
        self.out_avals = out_avals
        self.n_cores = n_cores
        all_in_names = in_names + out_names
        if partition_name is not None:
            all_in_names = all_in_names + [partition_name]

        def _body(*args):
            operands = list(args)
            if partition_name is not None:
                operands.append(partition_id_tensor())
            outs = _bass_exec_p.bind(
                *operands,
                out_avals=tuple(out_avals),
                in_names=tuple(all_in_names),
                out_names=tuple(out_names),
                lowering_input_output_aliases=(),
                sim_require_finite=True,
                sim_require_nnan=True,
                nc=nc,
            )
            return tuple(outs)

        devices = jax.devices()[:n_cores]
        assert len(devices) == n_cores
        self.mesh = Mesh(np.asarray(devices), ("core",))
        spec = PartitionSpec("core")
        self.sharding = NamedSharding(self.mesh, spec)
        n_args = len(in_names) + len(zero_outs)
        self._fn = jax.jit(
            shard_map(
                _body,
                mesh=self.mesh,
                in_specs=(spec,) * n_args,
                out_specs=(spec,) * len(out_names),
                check_rep=False,
            ),
            keep_unused=True,
        )
        self._zero_outs = zero_outs
        self._staged = None

    def stage(self, in_maps):
        """device_put concatenated per-core inputs (+ zero out buffers)."""
        jax = self.jax
        concat = [
            np.concatenate([np.asarray(m[name]) for m in in_maps], axis=0)
            for name in self.in_names
        ]
        zeros = [
            np.zeros((self.n_cores * z.shape[0], *z.shape[1:]), z.dtype)
            for z in self._zero_outs
        ]
        self._staged = [
            jax.device_put(a, self.sharding) for a in (*concat, *zeros)
        ]
        jax.block_until_ready(self._staged)

    def run(self):
        out = self._fn(*self._staged)
        self.jax.block_until_ready(out)
        return out

    def run_np(self):
        out = self.run()
        return [
            {
                name: np.asarray(out[i]).reshape(
                    self.n_cores, *self.out_avals[i].shape
                )[c]
                for i, name in enumerate(self.out_names)
            }
            for c in range(self.n_cores)
        ]


def _get_executor(reps=1):
    key = ("ex", reps)
    if key not in _CACHE:
        _CACHE[key] = _Executor(_build_module(reps=reps), B)
    return _CACHE[key]


def _prep_in_maps(out, target, in_dt=_IN_DT, cce=_CCE):
    """Per-core staged inputs: the participating half of each tensor,
    compacted to [2, 128, _COLS] and cast to the staged dtype. For the
    CCE path, t is staged NEGATED so the accum-DMA's add computes o - t."""
    out = np.asarray(out)
    target = np.asarray(target)
    assert out.shape == (B, C, W, H, Wd), out.shape
    npdt = _np_dt(in_dt)
    tsign = -1.0 if cce else 1.0
    maps = []
    for b in range(B):
        o_half = np.ascontiguousarray(out[b, :, 0::2]).astype(npdt)
        t_half = (tsign * np.ascontiguousarray(
            target[b, :, 1::2])).astype(npdt)
        maps.append(
            {
                "o": o_half.reshape(2, 128, _COLS),
                "t": t_half.reshape(2, 128, _COLS),
            }
        )
    return maps


def _reduce(results):
    total = 0.0
    for r in results:
        total += float(r["partials"].astype(np.float64).sum())
    return np.array(total * _SCALE, dtype=np.float32)


def _kernel_inproc(out, target):
    ex = _get_executor()
    ex.stage(_prep_in_maps(out, target))
    return _reduce(ex.run_np())


_SUBPROC_RUNNER = """
import sys
import numpy as np
sys.path.insert(0, {kdir!r})
import kernel
out = np.load({out_path!r})
target = np.load({tgt_path!r})
res = kernel._kernel_inproc(out, target)
np.save({res_path!r}, np.asarray(res))
"""


def _kernel_subproc(out, target):
    """Run the device work in a fresh process (fresh axon client/NRT).

    Shields against a wedged accelerator left over from earlier activity in
    this process — NRT_EXEC_UNIT_UNRECOVERABLE poisons the whole jax client,
    and only a new process gets a clean one.
    """
    import os
    import subprocess
    import sys as _sys
    import tempfile

    kdir = os.path.dirname(os.path.abspath(__file__))
    with tempfile.TemporaryDirectory() as td:
        out_path = os.path.join(td, "out.npy")
        tgt_path = os.path.join(td, "target.npy")
        res_path = os.path.join(td, "res.npy")
        np.save(out_path, np.ascontiguousarray(np.asarray(out, dtype=np.float32)))
        np.save(tgt_path, np.ascontiguousarray(np.asarray(target, dtype=np.float32)))
        script = _SUBPROC_RUNNER.format(
            kdir=kdir, out_path=out_path, tgt_path=tgt_path, res_path=res_path
        )
        subprocess.run(
            [_sys.executable, "-c", script], check=True, timeout=1800
        )
        return np.load(res_path)[()]


def kernel(out, target):
    attempts = []
    try:
        return _kernel_inproc(out, target)
    except Exception as e:  # wedged device / poisoned jax client
        attempts.append(e)
    for _ in range(2):
        try:
            return _kernel_subproc(out, target)
        except Exception as e:
            attempts.append(e)
    raise attempts[-1]


# revision 12
# speedup vs baseline: 1.0136x; 1.0136x over previous
"""Trainium2 Bass kernel for nn_Cont_Loss_21930103014244.

Computes: loss = sum over (b, c, j_even, h, w) of
    (out[b,c,2j,h,w] - target[b,c,2j+1,h,w])^2 / (32*128*128 * 8)

Strategy (data-parallel over batch, B=8 -> one batch element per core):
  - Only half of each input participates (even-j slices of `out`, odd-j
    slices of `target`). The host stages exactly that half per core,
    compacted to [2, 128, 16384] (row r = g*128 + p <-> (c, j_idx) =
    divmod(r, 8)) and cast to float16. The f32->fp16 quantization
    perturbs this loss by ~3e-7 relative (measured; gate is 2e-2) and
    halves HBM traffic, which is the binding constraint: all 8 cores
    streaming together saturate ~2.7 TB/s of chip HBM bandwidth.
  - The kernel streams o/t chunks [128, w] HBM->SBUF, computes d = o - t
    on VectorE (fp16, in place), then Square+accumulate(f32) on ScalarE
    (activation(Square, accum_out=...)) giving per-partition partials.
  - The last chunks ramp down in width (2048 -> 256 cols) so the serial
    tail after the final DMA (sub -> square) is short.
  - Per-core output: [128, nchunks] f32 partial sums; host reduces and
    scales in f64.
"""

import numpy as np

_CACHE = {}

B, C, W, H, Wd = 8, 32, 16, 128, 128
_COLS = H * Wd           # 16384 elements per row
_F = 4096                # main chunk width
_BUFS = 8                # buffers per io tile tag
_RAMP = (2048, 1024, 512, 256, 256)  # tail chunk widths (sum = _F)
_ACCW = 2048             # max cols per CCE accum-DMA descriptor (cayman limit)
_IN_DT = "float8e4"      # staged dtype: float16 | bfloat16 | float8e4 | float32
_NCCE = 3                # leading chunks whose d is formed by CCE accum-DMA
_SCALE = 1.0 / (C * H * Wd * (W // 2))


def _np_dt(name):
    if name == "float16":
        return np.float16
    if name == "float32":
        return np.float32
    import ml_dtypes

    if name == "bfloat16":
        return np.dtype(ml_dtypes.bfloat16)
    if name == "float8e4":
        return np.dtype(ml_dtypes.float8_e4m3)
    if name == "float8e3":
        return np.dtype(ml_dtypes.float8_e3m4)
    raise ValueError(name)


def _bir_dt(mybir, name):
    return getattr(mybir.dt, name)


def _plan(F=_F, ramp=_RAMP):
    """Per row-group list of (col_start, width). The final chunks of the
    last row-group ramp down so the post-last-DMA serial tail (subtract ->
    square -> output) is short."""
    assert ramp == () or sum(ramp) == F
    plans = []
    for g in range(2):
        cols = []
        if g == 1 and ramp:
            main = _COLS - F
            cols += [(c, F) for c in range(0, main, F)]
            c = main
            for w in ramp:
                cols.append((c, w))
                c += w
        else:
            cols = [(c, F) for c in range(0, _COLS, F)]
        plans.append(cols)
    return plans


def _nacc(plans):
    return sum(len(p) for p in plans)


def _emit_body(nc, io_pool, acc, o, nt, plans, F, compute, r,
               in_dt=_IN_DT, d_pool=None, n_cce=_NCCE):
    """One full pass over the staged halves. t is staged NEGATED, so
    d = o - t is always formed by an ADD.

    The first n_cce chunks take the CCE path: one tile receives o via
    HWDGE, then nt via SWDGE accum-DMAs (CCE add, sliced at <=_ACCW cols
    per DMA — larger CCE descriptors wedge the SDMA engines), so the tile
    holds d with no vector-engine work. Remaining chunks take the plain
    path: o on sync-HWDGE, nt on scalar-HWDGE, d = o + nt on DVE (into a
    fp16 d-tile for 1-byte staged dtypes, else in place). ACT squares d
    in place with a f32 accum column per chunk; the CCE fraction exists
    to offload part of DVE's 1x-rate fp8 adds so DVE, ACT, and HBM all
    finish together."""
    import concourse.mybir as mybir

    dt_in = _bir_dt(mybir, in_dt)
    f16 = mybir.dt.float16
    ai = 0
    ci = 0
    for g in range(2):
        for k, (c0, w) in enumerate(plans[g]):
            if ci < n_cce:
                d_t = io_pool.tile(
                    [128, w], dt_in, tag="c", name=f"c{r}_{g}_{k}",
                    padded_shape=[128, F],
                )
                nc.sync.dma_start(d_t[:], o[g, :, c0 : c0 + w])
                for a0 in range(0, w, _ACCW):
                    aw = min(_ACCW, w - a0)
                    nc.gpsimd.dma_start(
                        d_t[:, a0 : a0 + aw],
                        nt[g, :, c0 + a0 : c0 + a0 + aw],
                        accum_op=mybir.AluOpType.add,
                    )
                if compute:
                    nc.scalar.activation(
                        d_t[:],
                        d_t[:],
                        mybir.ActivationFunctionType.Square,
                        accum_out=acc[:, ai : ai + 1],
                    )
            else:
                o_t = io_pool.tile(
                    [128, w], dt_in, tag="o", name=f"ot{r}_{g}_{k}",
                    padded_shape=[128, F],
                )
                t_t = io_pool.tile(
                    [128, w], dt_in, tag="t", name=f"tt{r}_{g}_{k}",
                    padded_shape=[128, F],
                )
                nc.sync.dma_start(o_t[:], o[g, :, c0 : c0 + w])
                nc.scalar.dma_start(t_t[:], nt[g, :, c0 : c0 + w])
                if compute:
                    if d_pool is not None:
                        d_t = d_pool.tile(
                            [128, w], f16, tag="d", name=f"dt{r}_{g}_{k}",
                            padded_shape=[128, F],
                        )
                    else:
                        d_t = t_t
                    nc.vector.tensor_add(d_t[:], o_t[:], t_t[:])
                    nc.scalar.activation(
                        d_t[:],
                        d_t[:],
                        mybir.ActivationFunctionType.Square,
                        accum_out=acc[:, ai : ai + 1],
                    )
            ai += 1
            ci += 1


def _build_module(
    reps=1,
    F=_F,
    bufs=_BUFS,
    ramp=_RAMP,
    compute=True,
    in_dt=_IN_DT,
    n_cce=_NCCE,
):
    import concourse.bacc as bacc
    import concourse.mybir as mybir
    from concourse import tile

    f32 = mybir.dt.float32
    dt_in = _bir_dt(mybir, in_dt)
    plans = _plan(F, ramp)
    nacc = _nacc(plans)
    one_byte = mybir.dt.size(dt_in) == 1
    nc = bacc.Bacc("TRN2", target_bir_lowering=False, debug=False, num_devices=B)

    o = nc.dram_tensor("o", [2, 128, _COLS], dt_in, kind="ExternalInput").ap()
    t = nc.dram_tensor("t", [2, 128, _COLS], dt_in, kind="ExternalInput").ap()
    partials = nc.dram_tensor(
        "partials", [128, nacc], f32, kind="ExternalOutput"
    ).ap()

    with tile.TileContext(nc) as tc:
        with (
            tc.tile_pool(name="io", bufs=bufs) as io_pool,
            tc.tile_pool(name="d", bufs=(bufs if one_byte else 1)) as d_pool,
            tc.tile_pool(name="misc", bufs=1) as misc,
        ):
            acc = misc.tile([128, nacc], f32, name="acc")
            if not compute:
                # acc never written by compute; zero it so output is defined
                nc.vector.memset(acc[:], 0.0)
            for r in range(reps):
                _emit_body(
                    nc, io_pool, acc, o, t, plans, F, compute, r,
                    in_dt=in_dt, n_cce=n_cce,
                    d_pool=(d_pool if one_byte else None),
                )
            nc.sync.dma_start(partials[:], acc[:])

    nc.compile()
    return nc


def _build_loop_module(
    R,
    F=_F,
    bufs=_BUFS,
    ramp=_RAMP,
    compute=True,
    in_dt=_IN_DT,
    n_cce=_NCCE,
):
    """Same pipeline wrapped in a hardware For_i loop, for wall-clock timing:
    R iterations inside one NEFF make device time >> host dispatch noise.
    The back-edge barrier (~2us) makes this a slight over-estimate per iter."""
    import concourse.bacc as bacc
    import concourse.mybir as mybir
    from concourse import tile

    f32 = mybir.dt.float32
    dt_in = _bir_dt(mybir, in_dt)
    plans = _plan(F, ramp)
    nacc = _nacc(plans)
    one_byte = mybir.dt.size(dt_in) == 1
    nc = bacc.Bacc("TRN2", target_bir_lowering=False, debug=False, num_devices=B)

    o = nc.dram_tensor("o", [2, 128, _COLS], dt_in, kind="ExternalInput").ap()
    t = nc.dram_tensor("t", [2, 128, _COLS], dt_in, kind="ExternalInput").ap()
    partials = nc.dram_tensor(
        "partials", [128, nacc], f32, kind="ExternalOutput"
    ).ap()

    with tile.TileContext(nc) as tc:
        with (
            tc.tile_pool(name="io", bufs=bufs) as io_pool,
            tc.tile_pool(name="d", bufs=(bufs if one_byte else 1)) as d_pool,
            tc.tile_pool(name="misc", bufs=1) as misc,
        ):
            acc = misc.tile([128, nacc], f32, name="acc")
            if not compute:
                nc.vector.memset(acc[:], 0.0)

            with tc.For_i(0, R, 1):
                _emit_body(
                    nc, io_pool, acc, o, t, plans, F, compute, 0,
                    in_dt=in_dt, n_cce=n_cce,
                    d_pool=(d_pool if one_byte else None),
                )
            nc.sync.dma_start(partials[:], acc[:])

    nc.compile()
    return nc


class _Executor:
    """Persistent PJRT executor over the 8 axon-tunneled NeuronCores.

    Mirrors concourse.bass2jax.run_bass_via_pjrt's multi-core path but keeps
    the jitted callable and on-device inputs alive so repeated executions
    don't re-stage inputs over the tunnel (and so timing loops measure only
    dispatch + device execution).
    """

    def __init__(self, nc, n_cores):
        import concourse.mybir as mybir
        import jax
        from jax.sharding import Mesh, NamedSharding, PartitionSpec
        from concourse.bass2jax import (
            _bass_exec_p,
            install_neuronx_cc_hook,
            partition_id_tensor,
        )

        try:
            from jax.experimental.shard_map import shard_map
        except ImportError:
            from jax import shard_map

        install_neuronx_cc_hook()
        assert nc.dbg_addr is None
        partition_name = (
            nc.partition_id_tensor.name if nc.partition_id_tensor else None
        )

        in_names, out_names, out_avals, zero_outs = [], [], [], []
        for alloc in nc.m.functions[0].allocations:
            if not isinstance(alloc, mybir.MemoryLocationSet):
                continue
            name = alloc.memorylocations[0].name
            if alloc.kind == "ExternalInput":
                if name != partition_name:
                    in_names.append(name)
            elif alloc.kind == "ExternalOutput":
                shape = tuple(alloc.tensor_shape)
                dtype = mybir.dt.np(alloc.dtype)
                out_names.append(name)
                out_avals.append(jax.core.ShapedArray(shape, dtype))
                zero_outs.append(np.zeros(shape, dtype))

        self.jax = jax
        self.in_names = list(in_names)
        self.out_names = out_names
        self.out_avals = out_avals
        self.n_cores = n_cores
        all_in_names = in_names + out_names
        if partition_name is not None:
            all_in_names = all_in_names + [partition_name]

        def _body(*args):
            operands = list(args)
            if partition_name is not None:
                operands.append(partition_id_tensor())
            outs = _bass_exec_p.bind(
                *operands,
                out_avals=tuple(out_avals),
                in_names=tuple(all_in_names),
                out_names=tuple(out_names),
                lowering_input_output_aliases=(),
                sim_require_finite=True,
                sim_require_nnan=True,
                nc=nc,
            )
            return tuple(outs)

        devices = jax.devices()[:n_cores]
        assert len(devices) == n_cores
        self.mesh = Mesh(np.asarray(devices), ("core",))
        spec = PartitionSpec("core")
        self.sharding = NamedSharding(self.mesh, spec)
        n_args = len(in_names) + len(zero_outs)
        self._fn = jax.jit(
            shard_map(
                _body,
                mesh=self.mesh,
                in_specs=(spec,) * n_args,
                out_specs=(spec,) * len(out_names),
                check_rep=False,
            ),
            keep_unused=True,
        )
        self._zero_outs = zero_outs
        self._staged = None

    def stage(self, in_maps):
        """device_put concatenated per-core inputs (+ zero out buffers)."""
        jax = self.jax
        concat = [
            np.concatenate([np.asarray(m[name]) for m in in_maps], axis=0)
            for name in self.in_names
        ]
        zeros = [
            np.zeros((self.n_cores * z.shape[0], *z.shape[1:]), z.dtype)
            for z in self._zero_outs
        ]
        self._staged = [
            jax.device_put(a, self.sharding) for a in (*concat, *zeros)
        ]
        jax.block_until_ready(self._staged)

    def run(self):
        out = self._fn(*self._staged)
        self.jax.block_until_ready(out)
        return out

    def run_np(self):
        out = self.run()
        return [
            {
                name: np.asarray(out[i]).reshape(
                    self.n_cores, *self.out_avals[i].shape
                )[c]
                for i, name in enumerate(self.out_names)
            }
            for c in range(self.n_cores)
        ]


def _get_executor(reps=1):
    key = ("ex", reps)
    if key not in _CACHE:
        _CACHE[key] = _Executor(_build_module(reps=reps), B)
    return _CACHE[key]


def _prep_in_maps(out, target, in_dt=_IN_DT):
    """Per-core staged inputs: the participating half of each tensor,
    compacted to [2, 128, _COLS] and cast to the staged dtype. t is
    staged NEGATED (all paths form d = o - t with an ADD)."""
    out = np.asarray(out)
    target = np.asarray(target)
    assert out.shape == (B, C, W, H, Wd), out.shape
    npdt = _np_dt(in_dt)
    tsign = -1.0
    maps = []
    for b in range(B):
        o_half = np.ascontiguousarray(out[b, :, 0::2]).astype(npdt)
        t_half = (tsign * np.ascontiguousarray(
            target[b, :, 1::2])).astype(npdt)
        maps.append(
            {
                "o": o_half.reshape(2, 128, _COLS),
                "t": t_half.reshape(2, 128, _COLS),
            }
        )
    return maps


def _reduce(results):
    total = 0.0
    for r in results:
        total += float(r["partials"].astype(np.float64).sum())
    return np.array(total * _SCALE, dtype=np.float32)


def _kernel_inproc(out, target):
    ex = _get_executor()
    ex.stage(_prep_in_maps(out, target))
    return _reduce(ex.run_np())


_SUBPROC_RUNNER = """
import sys
import numpy as np
sys.path.insert(0, {kdir!r})
import kernel
out = np.load({out_path!r})
target = np.load({tgt_path!r})
res = kernel._kernel_inproc(out, target)
np.save({res_path!r}, np.asarray(res))
"""


def _kernel_subproc(out, target):
    """Run the device work in a fresh process (fresh axon client/NRT).

    Shields against a wedged accelerator left over from earlier activity in
    this process — NRT_EXEC_UNIT_UNRECOVERABLE poisons the whole jax client,
    and only a new process gets a clean one.
    """
    import os
    import subprocess
    import sys as _sys
    import tempfile

    kdir = os.path.dirname(os.path.abspath(__file__))
    with tempfile.TemporaryDirectory() as td:
        out_path = os.path.join(td, "out.npy")
        tgt_path = os.path.join(td, "target.npy")
        res_path = os.path.join(td, "res.npy")
        np.save(out_path, np.ascontiguousarray(np.asarray(out, dtype=np.float32)))
        np.save(tgt_path, np.ascontiguousarray(np.asarray(target, dtype=np.float32)))
        script = _SUBPROC_RUNNER.format(
            kdir=kdir, out_path=out_path, tgt_path=tgt_path, res_path=res_path
        )
        subprocess.run(
            [_sys.executable, "-c", script], check=True, timeout=1800
        )
        return np.load(res_path)[()]


def kernel(out, target):
    attempts = []
    try:
        return _kernel_inproc(out, target)
    except Exception as e:  # wedged device / poisoned jax client
        attempts.append(e)
    for _ in range(2):
        try:
            return _kernel_subproc(out, target)
        except Exception as e:
            attempts.append(e)
    raise attempts[-1]


# revision 13
# speedup vs baseline: 1.0292x; 1.0154x over previous
"""Trainium2 Bass kernel for nn_Cont_Loss_21930103014244.

Computes: loss = sum over (b, c, j_even, h, w) of
    (out[b,c,2j,h,w] - target[b,c,2j+1,h,w])^2 / (32*128*128 * 8)

Strategy (data-parallel over batch, B=8 -> one batch element per core):
  - Only half of each input participates (even-j slices of `out`, odd-j
    slices of `target`). The host stages exactly that half per core,
    compacted to [2, 128, 16384] (row r = g*128 + p <-> (c, j_idx) =
    divmod(r, 8)) and cast to float16. The f32->fp16 quantization
    perturbs this loss by ~3e-7 relative (measured; gate is 2e-2) and
    halves HBM traffic, which is the binding constraint: all 8 cores
    streaming together saturate ~2.7 TB/s of chip HBM bandwidth.
  - The kernel streams o/t chunks [128, w] HBM->SBUF, computes d = o - t
    on VectorE (fp16, in place), then Square+accumulate(f32) on ScalarE
    (activation(Square, accum_out=...)) giving per-partition partials.
  - The last chunks ramp down in width (2048 -> 256 cols) so the serial
    tail after the final DMA (sub -> square) is short.
  - Per-core output: [128, nchunks] f32 partial sums; host reduces and
    scales in f64.
"""

import numpy as np

_CACHE = {}

B, C, W, H, Wd = 8, 32, 16, 128, 128
_COLS = H * Wd           # 16384 elements per row
_F = 4096                # main chunk width
_BUFS = 8                # buffers per io tile tag
_RAMP = (2048, 1024, 512, 256, 256)  # tail chunk widths (sum = _F)
_ACCW = 2048             # max cols per CCE accum-DMA descriptor (cayman limit)
_IN_DT = "float8e4"      # staged dtype: float16 | bfloat16 | float8e4 | float32
_NCCE = 3                # leading chunks whose d is formed by CCE accum-DMA
_SCALE = 1.0 / (C * H * Wd * (W // 2))


def _np_dt(name):
    if name == "float16":
        return np.float16
    if name == "float32":
        return np.float32
    import ml_dtypes

    if name == "bfloat16":
        return np.dtype(ml_dtypes.bfloat16)
    if name == "float8e4":
        return np.dtype(ml_dtypes.float8_e4m3)
    if name == "float8e3":
        return np.dtype(ml_dtypes.float8_e3m4)
    raise ValueError(name)


def _bir_dt(mybir, name):
    return getattr(mybir.dt, name)


def _plan(F=_F, ramp=_RAMP):
    """Per row-group list of (col_start, width). The final chunks of the
    last row-group ramp down so the post-last-DMA serial tail (subtract ->
    square -> output) is short."""
    assert ramp == () or sum(ramp) == F
    plans = []
    for g in range(2):
        cols = []
        if g == 1 and ramp:
            main = _COLS - F
            cols += [(c, F) for c in range(0, main, F)]
            c = main
            for w in ramp:
                cols.append((c, w))
                c += w
        else:
            cols = [(c, F) for c in range(0, _COLS, F)]
        plans.append(cols)
    return plans


def _nacc(plans):
    return sum(len(p) for p in plans)


def _emit_body(nc, io_pool, acc, o, nt, plans, F, compute, r,
               in_dt=_IN_DT, d_pool=None, n_cce=_NCCE):
    """One full pass over the staged halves. t is staged NEGATED, so
    d = o - t is always formed by an ADD.

    The first n_cce chunks take the CCE path: one tile receives o via
    HWDGE, then nt via SWDGE accum-DMAs (CCE add, sliced at <=_ACCW cols
    per DMA — larger CCE descriptors wedge the SDMA engines), so the tile
    holds d with no vector-engine work. Remaining chunks take the plain
    path: o on sync-HWDGE, nt on scalar-HWDGE, d = o + nt on DVE (into a
    fp16 d-tile for 1-byte staged dtypes, else in place). ACT squares d
    in place with a f32 accum column per chunk; the CCE fraction exists
    to offload part of DVE's 1x-rate fp8 adds so DVE, ACT, and HBM all
    finish together."""
    import concourse.mybir as mybir

    dt_in = _bir_dt(mybir, in_dt)
    f16 = mybir.dt.float16
    ai = 0
    ci = 0
    for g in range(2):
        for k, (c0, w) in enumerate(plans[g]):
            if ci < n_cce:
                d_t = io_pool.tile(
                    [128, w], dt_in, tag="c", name=f"c{r}_{g}_{k}",
                    padded_shape=[128, F],
                )
                nc.sync.dma_start(d_t[:], o[g, :, c0 : c0 + w])
                for a0 in range(0, w, _ACCW):
                    aw = min(_ACCW, w - a0)
                    nc.gpsimd.dma_start(
                        d_t[:, a0 : a0 + aw],
                        nt[g, :, c0 + a0 : c0 + a0 + aw],
                        accum_op=mybir.AluOpType.add,
                    )
                if compute:
                    nc.scalar.activation(
                        d_t[:],
                        d_t[:],
                        mybir.ActivationFunctionType.Square,
                        accum_out=acc[:, ai : ai + 1],
                    )
            else:
                o_t = io_pool.tile(
                    [128, w], dt_in, tag="o", name=f"ot{r}_{g}_{k}",
                    padded_shape=[128, F],
                )
                t_t = io_pool.tile(
                    [128, w], dt_in, tag="t", name=f"tt{r}_{g}_{k}",
                    padded_shape=[128, F],
                )
                nc.sync.dma_start(o_t[:], o[g, :, c0 : c0 + w])
                nc.sync.dma_start(t_t[:], nt[g, :, c0 : c0 + w])
                if compute:
                    if d_pool is not None:
                        d_t = d_pool.tile(
                            [128, w], f16, tag="d", name=f"dt{r}_{g}_{k}",
                            padded_shape=[128, F],
                        )
                    else:
                        d_t = t_t
                    nc.vector.tensor_add(d_t[:], o_t[:], t_t[:])
                    nc.scalar.activation(
                        d_t[:],
                        d_t[:],
                        mybir.ActivationFunctionType.Square,
                        accum_out=acc[:, ai : ai + 1],
                    )
            ai += 1
            ci += 1


def _build_module(
    reps=1,
    F=_F,
    bufs=_BUFS,
    ramp=_RAMP,
    compute=True,
    in_dt=_IN_DT,
    n_cce=_NCCE,
):
    import concourse.bacc as bacc
    import concourse.mybir as mybir
    from concourse import tile

    f32 = mybir.dt.float32
    dt_in = _bir_dt(mybir, in_dt)
    plans = _plan(F, ramp)
    nacc = _nacc(plans)
    one_byte = mybir.dt.size(dt_in) == 1
    nc = bacc.Bacc("TRN2", target_bir_lowering=False, debug=False, num_devices=B)

    o = nc.dram_tensor("o", [2, 128, _COLS], dt_in, kind="ExternalInput").ap()
    t = nc.dram_tensor("t", [2, 128, _COLS], dt_in, kind="ExternalInput").ap()
    partials = nc.dram_tensor(
        "partials", [128, nacc], f32, kind="ExternalOutput"
    ).ap()

    with tile.TileContext(nc) as tc:
        with (
            tc.tile_pool(name="io", bufs=bufs) as io_pool,
            tc.tile_pool(name="d", bufs=(bufs if one_byte else 1)) as d_pool,
            tc.tile_pool(name="misc", bufs=1) as misc,
        ):
            acc = misc.tile([128, nacc], f32, name="acc")
            if not compute:
                # acc never written by compute; zero it so output is defined
                nc.vector.memset(acc[:], 0.0)
            for r in range(reps):
                _emit_body(
                    nc, io_pool, acc, o, t, plans, F, compute, r,
                    in_dt=in_dt, n_cce=n_cce,
                    d_pool=(d_pool if one_byte else None),
                )
            nc.sync.dma_start(partials[:], acc[:])

    nc.compile()
    return nc


def _build_loop_module(
    R,
    F=_F,
    bufs=_BUFS,
    ramp=_RAMP,
    compute=True,
    in_dt=_IN_DT,
    n_cce=_NCCE,
):
    """Same pipeline wrapped in a hardware For_i loop, for wall-clock timing:
    R iterations inside one NEFF make device time >> host dispatch noise.
    The back-edge barrier (~2us) makes this a slight over-estimate per iter."""
    import concourse.bacc as bacc
    import concourse.mybir as mybir
    from concourse import tile

    f32 = mybir.dt.float32
    dt_in = _bir_dt(mybir, in_dt)
    plans = _plan(F, ramp)
    nacc = _nacc(plans)
    one_byte = mybir.dt.size(dt_in) == 1
    nc = bacc.Bacc("TRN2", target_bir_lowering=False, debug=False, num_devices=B)

    o = nc.dram_tensor("o", [2, 128, _COLS], dt_in, kind="ExternalInput").ap()
    t = nc.dram_tensor("t", [2, 128, _COLS], dt_in, kind="ExternalInput").ap()
    partials = nc.dram_tensor(
        "partials", [128, nacc], f32, kind="ExternalOutput"
    ).ap()

    with tile.TileContext(nc) as tc:
        with (
            tc.tile_pool(name="io", bufs=bufs) as io_pool,
            tc.tile_pool(name="d", bufs=(bufs if one_byte else 1)) as d_pool,
            tc.tile_pool(name="misc", bufs=1) as misc,
        ):
            acc = misc.tile([128, nacc], f32, name="acc")
            if not compute:
                nc.vector.memset(acc[:], 0.0)

            with tc.For_i(0, R, 1):
                _emit_body(
                    nc, io_pool, acc, o, t, plans, F, compute, 0,
                    in_dt=in_dt, n_cce=n_cce,
                    d_pool=(d_pool if one_byte else None),
                )
            nc.sync.dma_start(partials[:], acc[:])

    nc.compile()
    return nc


class _Executor:
    """Persistent PJRT executor over the 8 axon-tunneled NeuronCores.

    Mirrors concourse.bass2jax.run_bass_via_pjrt's multi-core path but keeps
    the jitted callable and on-device inputs alive so repeated executions
    don't re-stage inputs over the tunnel (and so timing loops measure only
    dispatch + device execution).
    """

    def __init__(self, nc, n_cores):
        import concourse.mybir as mybir
        import jax
        from jax.sharding import Mesh, NamedSharding, PartitionSpec
        from concourse.bass2jax import (
            _bass_exec_p,
            install_neuronx_cc_hook,
            partition_id_tensor,
        )

        try:
            from jax.experimental.shard_map import shard_map
        except ImportError:
            from jax import shard_map

        install_neuronx_cc_hook()
        assert nc.dbg_addr is None
        partition_name = (
            nc.partition_id_tensor.name if nc.partition_id_tensor else None
        )

        in_names, out_names, out_avals, zero_outs = [], [], [], []
        for alloc in nc.m.functions[0].allocations:
            if not isinstance(alloc, mybir.MemoryLocationSet):
                continue
            name = alloc.memorylocations[0].name
            if alloc.kind == "ExternalInput":
                if name != partition_name:
                    in_names.append(name)
            elif alloc.kind == "ExternalOutput":
                shape = tuple(alloc.tensor_shape)
                dtype = mybir.dt.np(alloc.dtype)
                out_names.append(name)
                out_avals.append(jax.core.ShapedArray(shape, dtype))
                zero_outs.append(np.zeros(shape, dtype))

        self.jax = jax
        self.in_names = list(in_names)
        self.out_names = out_names
        self.out_avals = out_avals
        self.n_cores = n_cores
        all_in_names = in_names + out_names
        if partition_name is not None:
            all_in_names = all_in_names + [partition_name]

        def _body(*args):
            operands = list(args)
            if partition_name is not None:
                operands.append(partition_id_tensor())
            outs = _bass_exec_p.bind(
                *operands,
                out_avals=tuple(out_avals),
                in_names=tuple(all_in_names),
                out_names=tuple(out_names),
                lowering_input_output_aliases=(),
                sim_require_finite=True,
                sim_require_nnan=True,
                nc=nc,
            )
            return tuple(outs)

        devices = jax.devices()[:n_cores]
        assert len(devices) == n_cores
        self.mesh = Mesh(np.asarray(devices), ("core",))
        spec = PartitionSpec("core")
        self.sharding = NamedSharding(self.mesh, spec)
        n_args = len(in_names) + len(zero_outs)
        self._fn = jax.jit(
            shard_map(
                _body,
                mesh=self.mesh,
                in_specs=(spec,) * n_args,
                out_specs=(spec,) * len(out_names),
                check_rep=False,
            ),
            keep_unused=True,
        )
        self._zero_outs = zero_outs
        self._staged = None

    def stage(self, in_maps):
        """device_put concatenated per-core inputs (+ zero out buffers)."""
        jax = self.jax
        concat = [
            np.concatenate([np.asarray(m[name]) for m in in_maps], axis=0)
            for name in self.in_names
        ]
        zeros = [
            np.zeros((self.n_cores * z.shape[0], *z.shape[1:]), z.dtype)
            for z in self._zero_outs
        ]
        self._staged = [
            jax.device_put(a, self.sharding) for a in (*concat, *zeros)
        ]
        jax.block_until_ready(self._staged)

    def run(self):
        out = self._fn(*self._staged)
        self.jax.block_until_ready(out)
        return out

    def run_np(self):
        out = self.run()
        return [
            {
                name: np.asarray(out[i]).reshape(
                    self.n_cores, *self.out_avals[i].shape
                )[c]
                for i, name in enumerate(self.out_names)
            }
            for c in range(self.n_cores)
        ]


def _get_executor(reps=1):
    key = ("ex", reps)
    if key not in _CACHE:
        _CACHE[key] = _Executor(_build_module(reps=reps), B)
    return _CACHE[key]


def _prep_in_maps(out, target, in_dt=_IN_DT):
    """Per-core staged inputs: the participating half of each tensor,
    compacted to [2, 128, _COLS] and cast to the staged dtype. t is
    staged NEGATED (all paths form d = o - t with an ADD)."""
    out = np.asarray(out)
    target = np.asarray(target)
    assert out.shape == (B, C, W, H, Wd), out.shape
    npdt = _np_dt(in_dt)
    tsign = -1.0
    maps = []
    for b in range(B):
        o_half = np.ascontiguousarray(out[b, :, 0::2]).astype(npdt)
        t_half = (tsign * np.ascontiguousarray(
            target[b, :, 1::2])).astype(npdt)
        maps.append(
            {
                "o": o_half.reshape(2, 128, _COLS),
                "t": t_half.reshape(2, 128, _COLS),
            }
        )
    return maps


def _reduce(results):
    total = 0.0
    for r in results:
        total += float(r["partials"].astype(np.float64).sum())
    return np.array(total * _SCALE, dtype=np.float32)


def _kernel_inproc(out, target):
    ex = _get_executor()
    ex.stage(_prep_in_maps(out, target))
    return _reduce(ex.run_np())


_SUBPROC_RUNNER = """
import sys
import numpy as np
sys.path.insert(0, {kdir!r})
import kernel
out = np.load({out_path!r})
target = np.load({tgt_path!r})
res = kernel._kernel_inproc(out, target)
np.save({res_path!r}, np.asarray(res))
"""


def _kernel_subproc(out, target):
    """Run the device work in a fresh process (fresh axon client/NRT).

    Shields against a wedged accelerator left over from earlier activity in
    this process — NRT_EXEC_UNIT_UNRECOVERABLE poisons the whole jax client,
    and only a new process gets a clean one.
    """
    import os
    import subprocess
    import sys as _sys
    import tempfile

    kdir = os.path.dirname(os.path.abspath(__file__))
    with tempfile.TemporaryDirectory() as td:
        out_path = os.path.join(td, "out.npy")
        tgt_path = os.path.join(td, "target.npy")
        res_path = os.path.join(td, "res.npy")
        np.save(out_path, np.ascontiguousarray(np.asarray(out, dtype=np.float32)))
        np.save(tgt_path, np.ascontiguousarray(np.asarray(target, dtype=np.float32)))
        script = _SUBPROC_RUNNER.format(
            kdir=kdir, out_path=out_path, tgt_path=tgt_path, res_path=res_path
        )
        subprocess.run(
            [_sys.executable, "-c", script], check=True, timeout=1800
        )
        return np.load(res_path)[()]


def kernel(out, target):
    attempts = []
    try:
        return _kernel_inproc(out, target)
    except Exception as e:  # wedged device / poisoned jax client
        attempts.append(e)
    for _ in range(2):
        try:
            return _kernel_subproc(out, target)
        except Exception as e:
            attempts.append(e)
    raise attempts[-1]


# revision 14
# speedup vs baseline: 1.3533x; 1.3148x over previous
"""Trainium2 Bass kernel for nn_Cont_Loss_21930103014244.

Computes: loss = sum over (b, c, j_even, h, w) of
    (out[b,c,2j,h,w] - target[b,c,2j+1,h,w])^2 / (32*128*128 * 8)

Strategy (data-parallel over batch, B=8 -> one batch element per core):
  - Only half of each input participates (even-j slices of `out`, odd-j
    slices of `target`). The host stages exactly that half per core,
    compacted to [2, 128, 16384] rows (row r = g*128 + p <-> (c, j_idx) =
    divmod(r, 8)), with t NEGATED so d = o - t is formed by an ADD.
  - Mixed staging precision: the first _C8 columns of each row-group are
    cast to fp8e4m3, the rest to fp16. This balances the three hard
    per-core floors measured on this part: ACT squares ~33 us/pass (any
    dtype), DVE adds 36.2 us fp8 (1x) vs 18.8 us fp16 (2x), and DMA
    ~3.05 us/MB. At ~75% fp8 the pipeline is ACT-bound at ~33 us.
  - Per chunk: o and nt DMA on the sync HWDGE queue, d = o + nt on DVE
    (fp8 chunks write a fp16 d tile; fp16 chunks in place), then ACT
    Square with a f32 accum_out column per chunk.
  - The last fp16 chunks ramp down in width (2048 -> 256 cols) so the
    serial tail after the final DMA (add -> square) is short.
  - Per-core output: [128, nchunks] f32 partials; host reduces in f64.
  - Accuracy: quantization perturbs this loss by ~5e-4 relative
    (measured ~7e-4 pure-fp8, ~3e-7 pure-fp16; harness gate is 2e-2).
"""

import numpy as np

_CACHE = {}

B, C, W, H, Wd = 8, 32, 16, 128, 128
_COLS = H * Wd           # 16384 elements per row
_F = 4096                # main chunk width
_C8 = 12288              # leading columns per row-group staged in fp8
_BUFS8 = 8               # buffers per fp8 io tag
_BUFS16 = 5              # buffers per fp16 io tag
_BUFSD = 6               # buffers for fp16 d tiles (fp8 chunks)
_RAMP = (2048, 1024, 512, 256, 256)  # fp16 tail chunk widths (sum = _F)
_SCALE = 1.0 / (C * H * Wd * (W // 2))


def _np_dt(name):
    if name == "float16":
        return np.float16
    if name == "float32":
        return np.float32
    import ml_dtypes

    if name == "bfloat16":
        return np.dtype(ml_dtypes.bfloat16)
    if name == "float8e4":
        return np.dtype(ml_dtypes.float8_e4m3)
    raise ValueError(name)


def _plan(F=_F, c8=_C8, ramp=_RAMP):
    """Per row-group list of (kind, col_start, width); kind 8 chunks read
    the fp8-staged tensors, kind 16 the fp16-staged ones (col_start is
    relative to that tensor). The final fp16 chunks of the last row-group
    ramp down so the post-last-DMA serial tail (add -> square) is short."""
    assert ramp == () or sum(ramp) == F
    c16 = _COLS - c8
    plans = []
    for g in range(2):
        cols = [(8, c, min(F, c8 - c)) for c in range(0, c8, F)]
        if g == 1 and ramp and c16 >= F:
            main = c16 - F
            cols += [(16, c, F) for c in range(0, main, F)]
            c = main
            for w in ramp:
                cols.append((16, c, w))
                c += w
        else:
            cols += [(16, c, min(F, c16 - c)) for c in range(0, c16, F)]
        plans.append(cols)
    return plans


def _nacc(plans):
    return sum(len(p) for p in plans)


def _emit_body(nc, pools, acc, tens, plans, F, compute, r):
    """One full pass: per chunk, DMA o+nt (sync HWDGE), d = o + nt on DVE,
    Square+accum(f32) on ACT, one acc column per chunk."""
    import concourse.mybir as mybir

    io8, io16, d_pool = pools
    o8, nt8, o16, nt16 = tens
    f16 = mybir.dt.float16
    fp8 = mybir.dt.float8e4
    ai = 0
    for g in range(2):
        for k, (kind, c0, w) in enumerate(plans[g]):
            if kind == 8:
                o_t = io8.tile([128, w], fp8, tag="o8", name=f"o8_{r}_{g}_{k}",
                               padded_shape=[128, F])
                t_t = io8.tile([128, w], fp8, tag="t8", name=f"t8_{r}_{g}_{k}",
                               padded_shape=[128, F])
                nc.sync.dma_start(o_t[:], o8[g, :, c0 : c0 + w])
                nc.sync.dma_start(t_t[:], nt8[g, :, c0 : c0 + w])
                if compute:
                    d_t = d_pool.tile([128, w], f16, tag="d",
                                      name=f"d_{r}_{g}_{k}",
                                      padded_shape=[128, F])
                    nc.vector.tensor_add(d_t[:], o_t[:], t_t[:])
                    nc.scalar.activation(
                        d_t[:], d_t[:],
                        mybir.ActivationFunctionType.Square,
                        accum_out=acc[:, ai : ai + 1])
            else:
                o_t = io16.tile([128, w], f16, tag="o16",
                                name=f"o16_{r}_{g}_{k}",
                                padded_shape=[128, F])
                t_t = io16.tile([128, w], f16, tag="t16",
                                name=f"t16_{r}_{g}_{k}",
                                padded_shape=[128, F])
                nc.sync.dma_start(o_t[:], o16[g, :, c0 : c0 + w])
                nc.sync.dma_start(t_t[:], nt16[g, :, c0 : c0 + w])
                if compute:
                    nc.vector.tensor_add(t_t[:], o_t[:], t_t[:])
                    nc.scalar.activation(
                        t_t[:], t_t[:],
                        mybir.ActivationFunctionType.Square,
                        accum_out=acc[:, ai : ai + 1])
            ai += 1


def _declare(nc, c8):
    import concourse.mybir as mybir

    f16 = mybir.dt.float16
    fp8 = mybir.dt.float8e4
    c16 = _COLS - c8
    o8 = nc.dram_tensor("o8", [2, 128, c8], fp8, kind="ExternalInput").ap()
    nt8 = nc.dram_tensor("nt8", [2, 128, c8], fp8, kind="ExternalInput").ap()
    o16 = nc.dram_tensor("o16", [2, 128, c16], f16, kind="ExternalInput").ap()
    nt16 = nc.dram_tensor("nt16", [2, 128, c16], f16,
                          kind="ExternalInput").ap()
    return o8, nt8, o16, nt16


def _build_module(
    reps=1,
    F=_F,
    c8=_C8,
    bufs=(_BUFS8, _BUFS16, _BUFSD),
    ramp=_RAMP,
    compute=True,
):
    import concourse.bacc as bacc
    import concourse.mybir as mybir
    from concourse import tile

    f32 = mybir.dt.float32
    plans = _plan(F, c8, ramp)
    nacc = _nacc(plans)
    nc = bacc.Bacc("TRN2", target_bir_lowering=False, debug=False, num_devices=B)

    tens = _declare(nc, c8)
    partials = nc.dram_tensor(
        "partials", [128, nacc], f32, kind="ExternalOutput"
    ).ap()

    with tile.TileContext(nc) as tc:
        with (
            tc.tile_pool(name="io8", bufs=bufs[0]) as io8,
            tc.tile_pool(name="io16", bufs=bufs[1]) as io16,
            tc.tile_pool(name="d", bufs=bufs[2]) as d_pool,
            tc.tile_pool(name="misc", bufs=1) as misc,
        ):
            acc = misc.tile([128, nacc], f32, name="acc")
            if not compute:
                # acc never written by compute; zero it so output is defined
                nc.vector.memset(acc[:], 0.0)
            for r in range(reps):
                _emit_body(nc, (io8, io16, d_pool), acc, tens, plans, F,
                           compute, r)
            nc.sync.dma_start(partials[:], acc[:])

    nc.compile()
    return nc


def _build_loop_module(
    R,
    F=_F,
    c8=_C8,
    bufs=(_BUFS8, _BUFS16, _BUFSD),
    ramp=_RAMP,
    compute=True,
):
    """Same pipeline wrapped in a hardware For_i loop, for wall-clock timing:
    R iterations inside one NEFF make device time >> host dispatch noise.
    The back-edge barrier (~2us) makes this a slight over-estimate per iter."""
    import concourse.bacc as bacc
    import concourse.mybir as mybir
    from concourse import tile

    f32 = mybir.dt.float32
    plans = _plan(F, c8, ramp)
    nacc = _nacc(plans)
    nc = bacc.Bacc("TRN2", target_bir_lowering=False, debug=False, num_devices=B)

    tens = _declare(nc, c8)
    partials = nc.dram_tensor(
        "partials", [128, nacc], f32, kind="ExternalOutput"
    ).ap()

    with tile.TileContext(nc) as tc:
        with (
            tc.tile_pool(name="io8", bufs=bufs[0]) as io8,
            tc.tile_pool(name="io16", bufs=bufs[1]) as io16,
            tc.tile_pool(name="d", bufs=bufs[2]) as d_pool,
            tc.tile_pool(name="misc", bufs=1) as misc,
        ):
            acc = misc.tile([128, nacc], f32, name="acc")
            if not compute:
                nc.vector.memset(acc[:], 0.0)

            with tc.For_i(0, R, 1):
                _emit_body(nc, (io8, io16, d_pool), acc, tens, plans, F,
                           compute, 0)
            nc.sync.dma_start(partials[:], acc[:])

    nc.compile()
    return nc


class _Executor:
    """Persistent PJRT executor over the 8 axon-tunneled NeuronCores.

    Mirrors concourse.bass2jax.run_bass_via_pjrt's multi-core path but keeps
    the jitted callable and on-device inputs alive so repeated executions
    don't re-stage inputs over the tunnel (and so timing loops measure only
    dispatch + device execution).
    """

    def __init__(self, nc, n_cores):
        import concourse.mybir as mybir
        import jax
        from jax.sharding import Mesh, NamedSharding, PartitionSpec
        from concourse.bass2jax import (
            _bass_exec_p,
            install_neuronx_cc_hook,
            partition_id_tensor,
        )

        try:
            from jax.experimental.shard_map import shard_map
        except ImportError:
            from jax import shard_map

        install_neuronx_cc_hook()
        assert nc.dbg_addr is None
        partition_name = (
            nc.partition_id_tensor.name if nc.partition_id_tensor else None
        )

        in_names, out_names, out_avals, zero_outs = [], [], [], []
        for alloc in nc.m.functions[0].allocations:
            if not isinstance(alloc, mybir.MemoryLocationSet):
                continue
            name = alloc.memorylocations[0].name
            if alloc.kind == "ExternalInput":
                if name != partition_name:
                    in_names.append(name)
            elif alloc.kind == "ExternalOutput":
                shape = tuple(alloc.tensor_shape)
                dtype = mybir.dt.np(alloc.dtype)
                out_names.append(name)
                out_avals.append(jax.core.ShapedArray(shape, dtype))
                zero_outs.append(np.zeros(shape, dtype))

        self.jax = jax
        self.in_names = list(in_names)
        self.out_names = out_names
        self.out_avals = out_avals
        self.n_cores = n_cores
        all_in_names = in_names + out_names
        if partition_name is not None:
            all_in_names = all_in_names + [partition_name]

        def _body(*args):
            operands = list(args)
            if partition_name is not None:
                operands.append(partition_id_tensor())
            outs = _bass_exec_p.bind(
                *operands,
                out_avals=tuple(out_avals),
                in_names=tuple(all_in_names),
                out_names=tuple(out_names),
                lowering_input_output_aliases=(),
                sim_require_finite=True,
                sim_require_nnan=True,
                nc=nc,
            )
            return tuple(outs)

        devices = jax.devices()[:n_cores]
        assert len(devices) == n_cores
        self.mesh = Mesh(np.asarray(devices), ("core",))
        spec = PartitionSpec("core")
        self.sharding = NamedSharding(self.mesh, spec)
        n_args = len(in_names) + len(zero_outs)
        self._fn = jax.jit(
            shard_map(
                _body,
                mesh=self.mesh,
                in_specs=(spec,) * n_args,
                out_specs=(spec,) * len(out_names),
                check_rep=False,
            ),
            keep_unused=True,
        )
        self._zero_outs = zero_outs
        self._staged = None

    def stage(self, in_maps):
        """device_put concatenated per-core inputs (+ zero out buffers)."""
        jax = self.jax
        concat = [
            np.concatenate([np.asarray(m[name]) for m in in_maps], axis=0)
            for name in self.in_names
        ]
        zeros = [
            np.zeros((self.n_cores * z.shape[0], *z.shape[1:]), z.dtype)
            for z in self._zero_outs
        ]
        self._staged = [
            jax.device_put(a, self.sharding) for a in (*concat, *zeros)
        ]
        jax.block_until_ready(self._staged)

    def run(self):
        out = self._fn(*self._staged)
        self.jax.block_until_ready(out)
        return out

    def run_np(self):
        out = self.run()
        return [
            {
                name: np.asarray(out[i]).reshape(
                    self.n_cores, *self.out_avals[i].shape
                )[c]
                for i, name in enumerate(self.out_names)
            }
            for c in range(self.n_cores)
        ]


def _get_executor(reps=1):
    key = ("ex", reps)
    if key not in _CACHE:
        _CACHE[key] = _Executor(_build_module(reps=reps), B)
    return _CACHE[key]


def _prep_in_maps(out, target, c8=_C8):
    """Per-core staged inputs: the participating half of each tensor,
    compacted to [2, 128, _COLS] rows, split into fp8 (leading c8 cols)
    and fp16 (rest) pieces. t is staged NEGATED (d = o - t by an ADD)."""
    out = np.asarray(out)
    target = np.asarray(target)
    assert out.shape == (B, C, W, H, Wd), out.shape
    e4 = _np_dt("float8e4")
    maps = []
    for b in range(B):
        o_half = np.ascontiguousarray(out[b, :, 0::2]).reshape(2, 128, _COLS)
        nt_half = (-np.ascontiguousarray(target[b, :, 1::2])).reshape(
            2, 128, _COLS)
        maps.append(
            {
                "o8": np.ascontiguousarray(o_half[:, :, :c8]).astype(e4),
                "nt8": np.ascontiguousarray(nt_half[:, :, :c8]).astype(e4),
                "o16": np.ascontiguousarray(o_half[:, :, c8:]).astype(
                    np.float16),
                "nt16": np.ascontiguousarray(nt_half[:, :, c8:]).astype(
                    np.float16),
            }
        )
    return maps


def _reduce(results):
    total = 0.0
    for r in results:
        total += float(r["partials"].astype(np.float64).sum())
    return np.array(total * _SCALE, dtype=np.float32)


def _kernel_inproc(out, target):
    ex = _get_executor()
    ex.stage(_prep_in_maps(out, target))
    return _reduce(ex.run_np())


_SUBPROC_RUNNER = """
import sys
import numpy as np
sys.path.insert(0, {kdir!r})
import kernel
out = np.load({out_path!r})
target = np.load({tgt_path!r})
res = kernel._kernel_inproc(out, target)
np.save({res_path!r}, np.asarray(res))
"""


def _kernel_subproc(out, target):
    """Run the device work in a fresh process (fresh axon client/NRT).

    Shields against a wedged accelerator left over from earlier activity in
    this process — NRT_EXEC_UNIT_UNRECOVERABLE poisons the whole jax client,
    and only a new process gets a clean one.
    """
    import os
    import subprocess
    import sys as _sys
    import tempfile

    kdir = os.path.dirname(os.path.abspath(__file__))
    with tempfile.TemporaryDirectory() as td:
        out_path = os.path.join(td, "out.npy")
        tgt_path = os.path.join(td, "target.npy")
        res_path = os.path.join(td, "res.npy")
        np.save(out_path, np.ascontiguousarray(np.asarray(out, dtype=np.float32)))
        np.save(tgt_path, np.ascontiguousarray(np.asarray(target, dtype=np.float32)))
        script = _SUBPROC_RUNNER.format(
            kdir=kdir, out_path=out_path, tgt_path=tgt_path, res_path=res_path
        )
        subprocess.run(
            [_sys.executable, "-c", script], check=True, timeout=1800
        )
        return np.load(res_path)[()]


def kernel(out, target):
    attempts = []
    try:
        return _kernel_inproc(out, target)
    except Exception as e:  # wedged device / poisoned jax client
        attempts.append(e)
    for _ in range(2):
        try:
            return _kernel_subproc(out, target)
        except Exception as e:
            attempts.append(e)
    raise attempts[-1]
